# revision 18
# baseline (speedup 1.0000x reference)
"""Trainium2 Bass kernel for nn_ExperimentalEncoder (GC-LSTM encoder + attention-LSTM decoder).

Self-contained: hardcodes B,S,N,F,H = 8,32,1024,4,128 and shards data-parallel
over batch across 8 NeuronCores (1 batch per core, no collectives).

Algebraic structure (validated against the reference numerics):
  - The reference returns the OLD cell state each encoder step, so cell == 0
    throughout: cnew = ig*cs, fg is dead.
  - Decoder softmax is over a size-1 axis == 1.0, so ctx = hseq.sum(S) is a
    constant: accumulate hsum during the encoder, never materialize hseq.
  - sigma cell order: position s holds cell perm[s], grouped by residue
    r = cell % 3: R0=[0,342) cells 0,3,..., R1=[342,683) cells 1,4,...,
    R2=[683,1024) cells 2,5,...  In this order the torch flat-3-chunk gate
    extraction becomes contiguous fp16 muls (DVE 2x mode) instead of
    stride-3 fp32 muls.  adj/x are host-side permuted to match; the output
    is un-permuted on the host.

Layouts on device (per core, feature-major: H on partitions, N on free dim):
  adjT16 (128, 8*1024) f16 : k-tile k cols [1024k,1024k+1024),
                             adjT16[p,1024k+n'] = adj[n', perm[128k+p]]
  hid    (128, 1024)   f16 : node(sigma)-major stationary for the adj matmul
  all matmuls fp16 inputs / fp32 PSUM accumulate; elementwise fp16 where
  contiguity allows DVE 2x, fp32 where precision needs it (cx, hsum).
"""
import numpy as np

import concourse.bacc as bacc
import concourse.tile as tile
from concourse import mybir
from concourse.bass_utils import run_bass_kernel_spmd

B, S, N, F, H = 8, 32, 1024, 4, 128
DEC_STEPS = 32
F16, F32 = mybir.dt.float16, mybir.dt.float32
AFT = mybir.ActivationFunctionType

# sigma regions: (base, count) for residue r = cell % 3
RB = ((0, 342), (342, 341), (683, 341))

# gate-psum "waves": per step, three [128,1024] f32 tiles (2 banks each)
# from a 2-slot pool, created/consumed in order j1 -> j2 -> j0.  Each tile
# holds gate j's 682/683 needed columns at base 0; matmul splits are
# bank-aligned so each bank has exactly one accumulation group
# (x-prefill carries start=True, the w1h matmul carries stop=True).
GMM = {  # j -> list of (dst_lo, dst_hi, n_lo, n_hi)
    0: [(0, 512, 342, 854), (512, 682, 854, 1024)],
    1: [(0, 512, 341, 853), (512, 683, 853, 1024)],
    2: [(0, 512, 341, 853), (512, 683, 853, 1024)],
}
# cs matmuls, strided rhs reads (src step 3, cells m = 3u + r), split over
# two 1-bank PSUM tiles csA/csB so ps_tr can share csA's slot:
# (tile_idx, dst_lo, cnt, src_start); cst dst col = 512*tile_idx + dst_lo
CSMM = [(0, 0, 342, 0), (0, 342, 170, 1), (1, 0, 171, 511), (1, 171, 341, 2)]

# sigmoid extractions: (name, wave j, lo, hi)
SIGS = [
    ("gs1", 1, 0, 683),      # j1 ig+og in one call
    ("gso2", 2, 341, 683),   # j2 og (342)
    ("gsa2", 2, 0, 341),     # j2 ig (341)
    ("gso0", 0, 341, 682),   # j0 og (341)
    ("gsa0", 0, 0, 341),     # j0 ig (341)
]
# cnew region r: (gs tile, src_lo, src_hi)
IGX = {0: ("gs1", 0, 342), 1: ("gsa2", 0, 341), 2: ("gsa0", 0, 341)}
# hnew region r: (gs tile, src_lo, src_hi)
OGX = {0: ("gso2", 0, 342), 1: ("gso0", 0, 341), 2: ("gs1", 342, 683)}


def build_program():
    nc = bacc.Bacc("TRN2", target_bir_lowering=False, debug=False)
    d_adjT = nc.dram_tensor("adjT", [128, 8 * N], F16, kind="ExternalInput")
    d_xb = nc.dram_tensor("xb", [128, 8 * S * F], F16, kind="ExternalInput")
    d_w1h = nc.dram_tensor("w1h", [128, 384], F16, kind="ExternalInput")
    d_w1x4 = nc.dram_tensor("w1x4", [128, 128], F16, kind="ExternalInput")
    d_w2h = nc.dram_tensor("w2h", [128, 128], F16, kind="ExternalInput")
    d_b1t = nc.dram_tensor("b1t", [128, 3], F32, kind="ExternalInput")
    d_wd = nc.dram_tensor("wd", [128, 1024], F16, kind="ExternalInput")
    d_id = nc.dram_tensor("ident", [128, 128], F32, kind="ExternalInput")
    d_out = nc.dram_tensor("out", [N, H], F32, kind="ExternalOutput")

    with tile.TileContext(nc) as tc:
        with tc.tile_pool(name="const", bufs=1) as cpool, \
             tc.tile_pool(name="state", bufs=1) as spool:
            adjT = cpool.tile([128, 8 * N], F16)
            xb = cpool.tile([128, S * F * 8], F16)
            w1h = cpool.tile([128, 384], F16)
            w1x4 = cpool.tile([128, 128], F16)
            w2h = cpool.tile([128, 128], F16)
            b1t = cpool.tile([128, 3], F32)
            wd = cpool.tile([128, 1024], F16)
            ident = cpool.tile([128, 128], F32)
            for t_, d_ in ((adjT, d_adjT), (xb, d_xb), (w1h, d_w1h),
                           (w1x4, d_w1x4), (w2h, d_w2h),
                           (b1t, d_b1t), (wd, d_wd), (ident, d_id)):
                nc.gpsimd.dma_start(t_[:], d_.ap())

            ident16 = spool.tile([128, 128], F16)
            nc.vector.tensor_copy(ident16[:], ident[:])
            hsum = spool.tile([128, N], F32)
            nc.vector.memset(hsum[:], 0.0)
            axt16 = spool.tile([128, N], F16)
            hsum16 = spool.tile([128, N], F16)
            hxf = spool.tile([128, N], F32)

            with tc.tile_pool(name="encps", bufs=1, space="PSUM") as eps, \
                 tc.tile_pool(name="encsb", bufs=2) as esb, \
                 tc.tile_pool(name="axsp", bufs=3) as axsp:
                # ---------------- phase A: AXT = (adj @ Xb).T ---------------
                axps = eps.tile([128, N], F32, tag="acc")
                for c in range(2):
                    for k in range(8):
                        nc.tensor.matmul(
                            axps[:, 512 * c:512 * c + 512],
                            xb[:, 128 * k:128 * k + 128],
                            adjT[:, 1024 * k + 512 * c:1024 * k + 512 * c + 512],
                            start=(k == 0), stop=(k == 7))
                nc.vector.tensor_copy(axt16[:], axps[:])

                axs = [None] * S

                def load_axs(t):
                    axs[t] = axsp.tile([128, N], F16, tag="axs",
                                       name=f"axs{t}")
                    for i in range(4):
                        nc.sync.dma_start(axs[t][32 * i:32 * i + 4, :],
                                          axt16[4 * t:4 * t + 4, :])

                load_axs(0)
                load_axs(1)

                psg_of = {}
                pscs_of = {}

                # x-side prefill: K=4 matmuls on disjoint PE row-groups.
                # Creates the wave tile for (t, j) and starts its banks.
                def x_pre_g(t, j, only):
                    ps = eps.tile([128, N], F32, tag="gw", name=f"gw{t}_{j}")
                    psg_of[(t, j)] = ps
                    row = {0: 0, 1: 32, 2: 96}[j]
                    for lo, hi, nlo, nhi in GMM[j]:
                        nc.tensor.matmul(ps[:, lo:hi], w1x4[row:row + 4, :],
                                         axs[t][row:row + 4, nlo:nhi],
                                         start=True, stop=only,
                                         tile_position=(row, 0))

                def x_pre_cs(t, only):
                    pscs_of[t] = (
                        eps.tile([128, 512], F32, tag="csA", name=f"csA{t}"),
                        eps.tile([128, 512], F32, tag="csB", name=f"csB{t}"),
                    )
                    for ti, lo, cnt, s0 in CSMM:
                        ps = pscs_of[t][ti]
                        nc.tensor.matmul(
                            ps[:, lo:lo + cnt], w1x4[64:68, :],
                            axs[t][64:68, s0:s0 + 3 * (cnt - 1) + 1:3],
                            start=(lo == 0), stop=(only and lo != 0),
                            tile_position=(64, 0))

                def g_mms(t, j):
                    ps = psg_of[(t, j)]
                    for lo, hi, nlo, nhi in GMM[j]:
                        nc.tensor.matmul(ps[:, lo:hi],
                                         w1h[:, 128 * j:128 * j + 128],
                                         ach[:, nlo:nhi],
                                         start=False, stop=True)

                def cs_mms(t):
                    for ti, lo, cnt, s0 in CSMM:
                        ps = pscs_of[t][ti]
                        nc.tensor.matmul(ps[:, lo:lo + cnt], w2h[:],
                                         ach[:, s0:s0 + 3 * (cnt - 1) + 1:3],
                                         start=False, stop=(lo != 0))

                # step 0: x-contributions only (hid == 0); the j0 wave for
                # step t is emitted inside iteration t (its slot frees only
                # after gs1(t) is read).
                x_pre_g(0, 1, True)
                x_pre_g(0, 2, True)
                x_pre_cs(0, True)

                # ---------------- encoder loop ------------------------------
                hid_cur = None
                ach = None
                for t in range(S):
                    last = t == S - 1
                    pscs = pscs_of[t]
                    gs = {}
                    cst = esb.tile([128, N], F16, tag="cst")
                    cnew = esb.tile([128, N], F16, tag="cnew")
                    tcn = esb.tile([128, N], F16, tag="tcn")
                    hnew = esb.tile([128, N], F16, tag="hnew")
                    if not last:
                        hid_nxt = esb.tile([128, N], F16, tag="hid")
                        ps_tr = eps.tile([128, N], F16, tag="csA",
                                         name=f"pstr{t}")
                        ps_ac = eps.tile([128, N], F32, tag="acc",
                                         name=f"psac{t}")

                    def sig(i):
                        nm, j, lo, hi = SIGS[i]
                        g = esb.tile([128, hi - lo], F16, tag=nm,
                                     name=f"{nm}_{t}")
                        nc.scalar.activation(g[:], psg_of[(t, j)][:, lo:hi],
                                             AFT.Sigmoid,
                                             bias=b1t[:, j:j + 1])
                        gs[nm] = g

                    def cs_tanh(ti):
                        # one call per psum half-tile: csA -> cst[0:512),
                        # csB -> cst[512:1024)
                        nc.scalar.activation(cst[:, 512 * ti:512 * ti + 512],
                                             pscs[ti][:], AFT.Tanh)

                    def cnew_mul(r):
                        b, c = RB[r]
                        nm, lo, hi = IGX[r]
                        nc.vector.tensor_mul(cnew[:, b:b + c],
                                             gs[nm][:, lo:hi], cst[:, b:b + c])

                    def tcn_tanh(part):
                        # part 0: regions R0+R1 = [0,683); part 1: R2
                        lo, hi = (0, 683) if part == 0 else (683, 1024)
                        nc.scalar.activation(tcn[:, lo:hi], cnew[:, lo:hi],
                                             AFT.Tanh)

                    def hnew_mul(r):
                        b, c = RB[r]
                        nm, lo, hi = OGX[r]
                        nc.vector.tensor_mul(hnew[:, b:b + c],
                                             gs[nm][:, lo:hi], tcn[:, b:b + c])

                    def trs(ks):
                        for k in ks:
                            sl = slice(128 * k, 128 * k + 128)
                            nc.tensor.transpose(ps_tr[:, sl], hnew[:, sl],
                                                ident16[:])

                    def adj_mms(ks):
                        for k in ks:
                            for c in range(2):
                                nc.tensor.matmul(
                                    ps_ac[:, 512 * c:512 * c + 512],
                                    hid_nxt[:, 128 * k:128 * k + 128],
                                    adjT[:, 1024 * k + 512 * c:
                                         1024 * k + 512 * c + 512],
                                    start=(k == 0), stop=(k == 7))

                    # ---- elementwise tail (t) interleaved with head (t+1)
                    sig(0)                               # gs1
                    # j0 gate wave of step t (slot frees once gs1(t) is read)
                    x_pre_g(t, 0, t == 0)
                    if t > 0:
                        g_mms(t, 0)
                    if not last:
                        x_pre_g(t + 1, 1, False)
                    cs_tanh(0)                           # cst[0:512)
                    cnew_mul(0)
                    sig(1)                               # gso2
                    sig(2)                               # gsa2
                    cs_tanh(1)                           # cst[512:1024)
                    cnew_mul(1)
                    sig(3)                               # gso0
                    tcn_tanh(0)                          # tcn[0:683)
                    hnew_mul(0)
                    if not last:
                        trs((0, 1))
                        nc.vector.tensor_copy(hid_nxt[:, 0:256],
                                              ps_tr[:, 0:256])
                        adj_mms((0, 1))
                        x_pre_g(t + 1, 2, False)
                    hnew_mul(1)
                    if not last:
                        trs((2, 3, 4))
                        nc.vector.tensor_copy(hid_nxt[:, 256:640],
                                              ps_tr[:, 256:640])
                        adj_mms((2, 3, 4))
                    sig(4)                               # gsa0
                    cnew_mul(2)
                    tcn_tanh(1)                          # tcn[683:1024)
                    hnew_mul(2)
                    if not last:
                        trs((5, 6, 7))
                        nc.vector.tensor_copy(hid_nxt[:, 640:1024],
                                              ps_tr[:, 640:1024])
                    nc.gpsimd.tensor_add(hsum[:], hsum[:], hnew[:])
                    if not last:
                        adj_mms((5, 6, 7))
                        x_pre_cs(t + 1, False)
                        ach = esb.tile([128, N], F16, tag="ach",
                                       name=f"ach{t+1}")
                        nc.vector.tensor_copy(ach[:, 341:1024],
                                              ps_ac[:, 341:1024])
                        nc.vector.tensor_copy(ach[:, 0:341], ps_ac[:, 0:341])
                        g_mms(t + 1, 1)
                        g_mms(t + 1, 2)
                        if t + 2 < S:
                            load_axs(t + 2)
                        cs_mms(t + 1)

            # ---------------- decoder --------------------------------------
            nc.vector.tensor_copy(hsum16[:], hsum[:])

            with tc.tile_pool(name="decps", bufs=2, space="PSUM") as dps, \
                 tc.tile_pool(name="decsb", bufs=2) as dsb, \
                 tc.tile_pool(name="decst", bufs=1) as dst:
                cx = dst.tile([128, N], F32)
                psd_of = {}

                def ctx_pre(t, h, only):
                    ps = dps.tile([128, 2048], F32, tag="gd",
                                  name=f"psd{t}_{h}")
                    psd_of[(t, h)] = ps
                    nsl = slice(512 * h, 512 * h + 512)
                    for g in range(4):
                        nc.tensor.matmul(
                            ps[:, 512 * g:512 * g + 512],
                            wd[:, 512 + 128 * g:512 + 128 * g + 128],
                            hsum16[:, nsl], start=True, stop=only)

                ctx_pre(0, 0, True)
                ctx_pre(0, 1, True)
                hx16 = None
                for t in range(DEC_STEPS):
                    first, last = t == 0, t == DEC_STEPS - 1
                    hx_n = (hxf if last
                            else dsb.tile([128, N], F16, tag="hx",
                                          name=f"hx{t}"))
                    for h in range(2):
                        nsl = slice(512 * h, 512 * h + 512)
                        psd = psd_of[(t, h)]
                        if not first:
                            for g in range(4):
                                nc.tensor.matmul(
                                    psd[:, 512 * g:512 * g + 512],
                                    wd[:, 128 * g:128 * g + 128],
                                    hx16[:, nsl], start=False, stop=True)
                        sg = dsb.tile([128, 1536], F16, tag=f"sg{h}",
                                      name=f"sg{t}_{h}")
                        tg = dsb.tile([128, 512], F16, tag=f"tg{h}",
                                      name=f"tg{t}_{h}")
                        nc.scalar.activation(sg[:], psd[:, 0:1536],
                                             AFT.Sigmoid)
                        nc.scalar.activation(tg[:], psd[:, 1536:2048],
                                             AFT.Tanh)
                        tcx = dsb.tile([128, 512], F16, tag=f"tcx{h}",
                                       name=f"tcx{t}_{h}")
                        if first:
                            nc.vector.tensor_mul(cx[:, nsl], sg[:, 0:512],
                                                 tg[:])
                        else:
                            m2 = dsb.tile([128, 512], F16, tag=f"m2{h}",
                                          name=f"m2_{t}_{h}")
                            m1 = dsb.tile([128, 512], F32, tag=f"m1{h}",
                                          name=f"m1_{t}_{h}")
                            nc.vector.tensor_mul(m2[:], sg[:, 0:512], tg[:])
                            nc.vector.tensor_mul(m1[:], sg[:, 512:1024],
                                                 cx[:, nsl])
                            nc.vector.tensor_add(cx[:, nsl], m1[:], m2[:])
                        nc.scalar.activation(tcx[:], cx[:, nsl], AFT.Tanh)
                        nc.vector.tensor_mul(hx_n[:, nsl], sg[:, 1024:1536],
                                             tcx[:])
                        if not last:
                            ctx_pre(t + 1, h, False)
                    hx16 = hx_n

            # ---------------- output transpose -----------------------------
            with tc.tile_pool(name="outps", bufs=2, space="PSUM") as ops, \
                 tc.tile_pool(name="outsb", bufs=1) as osb:
                out_sb = osb.tile([128, N], F32)
                for k in range(8):
                    pt = ops.tile([128, 128], F32, tag="tr")
                    nc.tensor.transpose(pt[:], hxf[:, 128 * k:128 * k + 128],
                                        ident[:])
                    nc.vector.tensor_copy(out_sb[:, 128 * k:128 * k + 128],
                                          pt[:])
                nc.sync.dma_start(
                    d_out.ap().rearrange("(k p) h -> p k h", p=128),
                    out_sb[:].rearrange("p (k h) -> p k h", k=8))
    nc.compile()
    return nc


_CACHE = {}


def _get_program():
    if "nc" not in _CACHE:
        _CACHE["nc"] = build_program()
    return _CACHE["nc"]


PERM = np.concatenate([np.arange(0, 1024, 3), np.arange(1, 1024, 3),
                       np.arange(2, 1024, 3)])


def _prep_in_maps(x, adj, W1, b1, W2, b2, W_ih, W_hh, b_ih, b_hh):
    f16, f32 = np.float16, np.float32
    A_sig = adj[:, PERM]                       # col s <-> source cell PERM[s]
    adjT16 = np.ascontiguousarray(
        A_sig.T.reshape(8, 128, N).transpose(1, 0, 2).reshape(128, 8 * N)
    ).astype(f16)
    w1h = W1[4:].astype(f16)
    w2h = W2[4:].astype(f16)
    w1x4 = np.zeros((128, 128), f16)
    w1x4[0:4] = W1[:4, 0:128].astype(f16)
    w1x4[32:36] = W1[:4, 128:256].astype(f16)
    w1x4[64:68] = W2[:4].astype(f16)
    w1x4[96:100] = W1[:4, 256:384].astype(f16)
    b1t = np.ascontiguousarray(b1.reshape(3, 128).T).astype(f32)
    reord = np.r_[0:128, 128:256, 384:512, 256:384]    # i,f,o,g
    wd = np.concatenate([W_hh[reord].T, W_ih[reord].T], axis=1).astype(f16)
    ident = np.eye(128, dtype=f32)
    common = dict(adjT=adjT16, w1h=w1h, w1x4=w1x4, w2h=w2h, b1t=b1t,
                  wd=wd, ident=ident)
    maps = []
    for b in range(B):
        xbn = x[b].transpose(1, 0, 2).reshape(N, S * F)[PERM]   # sigma rows
        xb16 = np.ascontiguousarray(
            xbn.reshape(8, 128, S * F).transpose(1, 0, 2).reshape(128, -1)
        ).astype(f16)
        maps.append(dict(common, xb=xb16))
    return maps


def run(inputs, trace=False):
    nc = _get_program()
    maps = _prep_in_maps(**{k: np.asarray(v) for k, v in inputs.items()})
    br = run_bass_kernel_spmd(nc, maps, list(range(B)), trace=trace)
    out_dev = np.stack([br.results[c]["out"] for c in range(B)])  # (B, N, H)
    out = np.empty_like(out_dev)
    out[:, PERM] = out_dev                                        # un-permute
    return out.astype(np.float32), br


def kernel(**inputs) -> np.ndarray:
    out, _ = run(inputs, trace=False)
    return out
